# revision 1
# baseline (speedup 1.0000x reference)
"""Trainium2 Bass kernel for CareGptOssAttentionHF (MLA-style sliding-window
attention with sinks).

Sharding: sequence-parallel across 8 NeuronCores. Core c owns query rows
[c*256, (c+1)*256) and redundantly computes latent/K/V for its 768-row key
halo [c*256-512, c*256+256) — no collectives needed (window = 512).

On-chip dataflow (per core, all fp16 operands, fp32 PSUM accumulation):
  qT   [d=128, h, i]   = WqT.T @ hsT_local (+bq), RoPE applied in partition dim
  latT [r=576, j]      = WkvaT.T @ hsT_halo (+bkva)
  kfT  [d=128, h, j]   : rows 0:64 = w_kc-projected nope, rows 64:128 = RoPE'd
                         shared k_rope broadcast per head
  v    [j, h*65+d]     = latT.T @ wvc (key-major; col 65 of each head = ones,
                         so the PV matmul's row 64 accumulates the softmax
                         denominator for free)
  scoresT[j, i] per (h, i-tile) -> exp(SCALE*x) on ACT (no max subtraction:
                         logits are O(5), fp32 exp is safe) -> multiplicative
                         {0,1} mask (host-computed, encodes causal+window+
                         sequence-start per core) -> PV matmul -> normalize by
                         1/(denom + exp(sink_h)) -> out = o_attnT.T @ WoT (+bo)
"""

import os
import sys

import numpy as np

if "/opt/trn_rl_repo" not in sys.path:
    sys.path.insert(0, "/opt/trn_rl_repo")

B, S, HID, H = 1, 2048, 2048, 16
NOPE = ROPE = 64
D = NOPE + ROPE  # 128
V = 64
R = 512
SW = 512
NCORES = 8
Q = S // NCORES  # 256 query rows per core
KH = Q + SW  # 768 halo key rows per core
NJB = KH // 128  # 6 key blocks
NIT = Q // 128  # 2 query tiles
SCALE = float(D) ** -0.5
DEBUG = bool(int(os.environ.get("BASSDBG", "0")))

_CACHE = {}


def _build_program():
    import concourse.bass as bass
    import concourse.mybir as mybir
    from concourse import tile
    from contextlib import ExitStack

    f32 = mybir.dt.float32
    f16 = mybir.dt.float16
    AF = mybir.ActivationFunctionType

    nc = bass.Bass()

    hsT_d = nc.dram_tensor("hsT", [HID, KH], f16, kind="ExternalInput")
    wqT_d = nc.dram_tensor("wqT", [HID, H * D], f16, kind="ExternalInput")
    wkvaT_d = nc.dram_tensor("wkvaT", [HID, R + ROPE], f16, kind="ExternalInput")
    wkc_d = nc.dram_tensor("wkc", [R, H * NOPE], f16, kind="ExternalInput")
    wvc_d = nc.dram_tensor("wvc", [R, H * V], f16, kind="ExternalInput")
    woT_d = nc.dram_tensor("woT", [H * V, HID], f16, kind="ExternalInput")
    bq_d = nc.dram_tensor("bq", [128, 16], f32, kind="ExternalInput")
    bkva_d = nc.dram_tensor("bkva", [128, 5], f32, kind="ExternalInput")
    bo_d = nc.dram_tensor("bo", [128, HID], f16, kind="ExternalInput")
    tqc_d = nc.dram_tensor("trigq_cos", [128, Q], f16, kind="ExternalInput")
    tqs_d = nc.dram_tensor("trigq_sin", [128, Q], f16, kind="ExternalInput")
    tk_d = nc.dram_tensor("trigk", [64, 2, KH], f16, kind="ExternalInput")
    mask_d = nc.dram_tensor("mask", [128, NIT, NJB, 128], f16, kind="ExternalInput")
    esink_d = nc.dram_tensor("esink", [128, H], f32, kind="ExternalInput")
    out_d = nc.dram_tensor("out", [Q, HID], f32, kind="ExternalOutput")

    dbg = {}
    if DEBUG:
        dbg["lat"] = nc.dram_tensor("dbg_lat", [128, 4, KH], f16, kind="ExternalOutput")
        dbg["lat4"] = nc.dram_tensor("dbg_lat4", [64, KH], f16, kind="ExternalOutput")
        dbg["q"] = nc.dram_tensor("dbg_q", [128, H, Q], f16, kind="ExternalOutput")
        dbg["kf"] = nc.dram_tensor("dbg_kf", [128, H, KH], f16, kind="ExternalOutput")
        dbg["v"] = nc.dram_tensor("dbg_v", [128, NJB, H * 2 * V], f16, kind="ExternalOutput")
        dbg["pr"] = nc.dram_tensor("dbg_pr", [128, H * NIT, KH], f16, kind="ExternalOutput")
        dbg["oat"] = nc.dram_tensor("dbg_oat", [128, 8, Q], f16, kind="ExternalOutput")

    with tile.TileContext(nc) as tc, ExitStack() as ctx:
        const = ctx.enter_context(tc.tile_pool(name="const", bufs=1))

        # ---- resident tiles + input DMAs ----
        hs_t = [const.tile([128, KH], f16, name=f"hs{k}") for k in range(16)]
        for k in range(16):
            nc.sync.dma_start(hs_t[k][:], hsT_d[k * 128 : (k + 1) * 128, :])
        wkva = const.tile([128, 16, R + ROPE], f16)
        for k in range(16):
            nc.sync.dma_start(wkva[:, k, :], wkvaT_d[k * 128 : (k + 1) * 128, :])
        wkc = const.tile([128, 4, H * NOPE], f16)
        for k in range(4):
            nc.sync.dma_start(wkc[:, k, :], wkc_d[k * 128 : (k + 1) * 128, :])
        wvc = const.tile([128, 4, H * V], f16)
        for k in range(4):
            nc.sync.dma_start(wvc[:, k, :], wvc_d[k * 128 : (k + 1) * 128, :])
        bq_sb = const.tile([128, 16], f32)
        nc.sync.dma_start(bq_sb[:], bq_d[:])
        bkva_sb = const.tile([128, 5], f32)
        nc.sync.dma_start(bkva_sb[:], bkva_d[:])
        bo_sb = const.tile([128, HID], f16)
        nc.sync.dma_start(bo_sb[:], bo_d[:])
        tqc = const.tile([128, Q], f16)
        nc.sync.dma_start(tqc[:], tqc_d[:])
        tqs = const.tile([128, Q], f16)
        nc.sync.dma_start(tqs[:], tqs_d[:])
        tk = const.tile([64, 2, KH], f16)
        nc.sync.dma_start(tk[:], tk_d[:])
        mask_sb = const.tile([128, NIT, NJB, 128], f16)
        nc.sync.dma_start(mask_sb[:], mask_d[:])
        esink_sb = const.tile([128, H], f32)
        nc.sync.dma_start(esink_sb[:], esink_d[:])
        # loaded last: first consumed by the final output projection
        wo_sb = const.tile([128, 8, HID], f16)
        for k in range(8):
            nc.sync.dma_start(wo_sb[:, k, :], woT_d[k * 128 : (k + 1) * 128, :])

        qT = const.tile([128, H, Q], f16)
        latbf = const.tile([128, 4, KH], f16)
        lat4 = const.tile([128, KH], f16)  # rows 0:64 = k_rope
        oat = const.tile([128, 8, Q], f16)
        rotq = const.tile([128, 8, Q], f16)
        rotk = const.tile([64, KH], f16)
        out_sb = const.tile([128, NIT, HID], f32)

        def bc(ap, n):
            # broadcast a [P, F] AP to [P, n, F] via a step-0 middle dim
            return bass.AP(ap.tensor, ap.offset, [ap.ap[0], [0, n], ap.ap[1]])

        def bcf(col, n):
            # broadcast a [P, 1] column AP to [P, n] via a step-0 free dim
            return bass.AP(col.tensor, col.offset, [col.ap[0], [0, n]])

        # ---- phase 1: q projection (feature-major) ----
        # Wq is loaded whole into an 8MB region with first-use (zero-wait)
        # DMAs; the region is released afterward and reused by kf/v_sb, whose
        # writers are compute engines (multi-wait capable), never DMAs —
        # DMA ring entries can carry at most ONE embedded wait.
        with tc.tile_pool(name="wqfull", bufs=1) as wqp, tc.tile_pool(
            name="psq", bufs=1, space="PSUM"
        ) as psqp:
            wq_t = [wqp.tile([128, H * D], f16, name=f"wq{k}") for k in range(16)]
            for k in range(16):
                nc.sync.dma_start(wq_t[k][:], wqT_d[k * 128 : (k + 1) * 128, :])
            for g in range(2):
                psq = [
                    psqp.tile([128, Q], f32, tag=f"psq{m}", name=f"psq{m}")
                    for m in range(8)
                ]
                for k in range(16):
                    for m in range(8):
                        fo = g * 1024 + m * 128
                        nc.tensor.matmul(
                            psq[m][:],
                            lhsT=wq_t[k][:, fo : fo + 128],
                            rhs=hs_t[k][:, SW:KH],
                            start=(k == 0),
                            stop=(k == 15),
                        )
                for m in range(8):
                    gm = g * 8 + m
                    nc.vector.tensor_add(
                        qT[:, gm, :], psq[m][:], bcf(bq_sb[:, gm : gm + 1], Q)
                    )

        # ---- phase 2: RoPE on q (rows 64:128 of each head tile) ----
        for hb in range(2):
            hs_ = slice(hb * 8, hb * 8 + 8)
            nc.vector.tensor_copy(rotq[64:96, :, :], qT[96:128, hs_, :])
            nc.vector.tensor_copy(rotq[96:128, :, :], qT[64:96, hs_, :])
            nc.vector.tensor_mul(qT[64:96, hs_, :], qT[64:96, hs_, :], bc(tqc[64:96, :], 8))
            nc.vector.tensor_mul(rotq[64:96, :, :], rotq[64:96, :, :], bc(tqs[64:96, :], 8))
            nc.vector.tensor_sub(qT[64:96, hs_, :], qT[64:96, hs_, :], rotq[64:96, :, :])
            nc.vector.tensor_mul(qT[96:128, hs_, :], qT[96:128, hs_, :], bc(tqc[96:128, :], 8))
            nc.vector.tensor_mul(rotq[96:128, :, :], rotq[96:128, :, :], bc(tqs[96:128, :], 8))
            nc.vector.tensor_add(qT[96:128, hs_, :], qT[96:128, hs_, :], rotq[96:128, :, :])

        # kf / v_sb reuse the released Wq region; their writers are ACT/DVE.
        kvp = ctx.enter_context(tc.tile_pool(name="kv", bufs=1))
        kf = kvp.tile([128, H, KH], f16)
        # per-head column layout [v(64) | ones(64)] so the PV lhsT slice is a
        # contiguous [128, 128] block whose second half replicates the denom
        v_sb = kvp.tile([128, NJB, H * 2 * V], f16)

        # ---- phase 3: latent projection ----
        with tc.tile_pool(name="pslat", bufs=1, space="PSUM") as pslatp:
            pslat = [
                pslatp.tile([128, KH], f32, tag=f"pslat{m}", name=f"pslat{m}")
                for m in range(4)
            ]
            for k in range(16):
                for m in range(4):
                    for n0, n1 in ((0, 512), (512, KH)):
                        nc.tensor.matmul(
                            pslat[m][:, n0:n1],
                            lhsT=wkva[:, k, m * 128 : (m + 1) * 128],
                            rhs=hs_t[k][:, n0:n1],
                            start=(k == 0),
                            stop=(k == 15),
                        )
            for m in range(4):
                nc.vector.tensor_add(
                    latbf[:, m, :], pslat[m][:], bcf(bkva_sb[:, m : m + 1], KH)
                )
            ps4 = pslatp.tile([64, KH], f32, tag="pslat0")
            for k in range(16):
                for n0, n1 in ((0, 512), (512, KH)):
                    nc.tensor.matmul(
                        ps4[:, n0:n1],
                        lhsT=wkva[:, k, 512:576],
                        rhs=hs_t[k][:, n0:n1],
                        start=(k == 0),
                        stop=(k == 15),
                    )
            nc.vector.tensor_add(lat4[0:64, :], ps4[:], bcf(bkva_sb[0:64, 4:5], KH))

        # ---- phase 4: RoPE on k_rope + broadcast into kf rows 64:128 ----
        nc.vector.tensor_copy(rotk[0:32, :], lat4[32:64, :])
        nc.vector.tensor_copy(rotk[32:64, :], lat4[0:32, :])
        nc.vector.tensor_mul(lat4[0:32, :], lat4[0:32, :], tk[0:32, 0, :])
        nc.vector.tensor_mul(rotk[0:32, :], rotk[0:32, :], tk[0:32, 1, :])
        nc.vector.tensor_sub(lat4[0:32, :], lat4[0:32, :], rotk[0:32, :])
        nc.vector.tensor_mul(lat4[32:64, :], lat4[32:64, :], tk[32:64, 0, :])
        nc.vector.tensor_mul(rotk[32:64, :], rotk[32:64, :], tk[32:64, 1, :])
        nc.vector.tensor_add(lat4[32:64, :], lat4[32:64, :], rotk[32:64, :])
        for h in range(H):
            nc.scalar.copy(kf[64:128, h, :], lat4[0:64, :])

        # ---- phase 5: k_nope into kf rows 0:64 ----
        with tc.tile_pool(name="pskn", bufs=4, space="PSUM") as psknp:
            for m in range(8):
                ps = psknp.tile([128, KH], f32, tag="pskn")
                for k in range(4):
                    for n0, n1 in ((0, 512), (512, KH)):
                        nc.tensor.matmul(
                            ps[:, n0:n1],
                            lhsT=wkc[:, k, m * 128 : (m + 1) * 128],
                            rhs=latbf[:, k, n0:n1],
                            start=(k == 0),
                            stop=(k == 3),
                        )
                nc.scalar.copy(kf[0:64, 2 * m, :], ps[0:64, :])
                nc.scalar.copy(kf[0:64, 2 * m + 1, :], ps[64:128, :])

        # ---- phase 6: V (key-major, with ones column per head) ----
        with tc.tile_pool(name="psv", bufs=2, space="PSUM") as psvp:
            for jb in range(NJB):
                ps = psvp.tile([128, H * V], f32, tag="psv")
                for k in range(4):
                    for n0, n1 in ((0, 512), (512, 1024)):
                        nc.tensor.matmul(
                            ps[:, n0:n1],
                            lhsT=latbf[:, k, jb * 128 : (jb + 1) * 128],
                            rhs=wvc[:, k, n0:n1],
                            start=(k == 0),
                            stop=(k == 3),
                        )
                vview = v_sb[:, jb, :].rearrange("p (h d) -> p h d", d=2 * V)
                ps_view = ps[:].rearrange("p (h d) -> p h d", d=V)
                nc.scalar.copy(vview[:, :, 0:V], ps_view)
                # constant 1.0 fill via native ACT (packed memset is a custom
                # DVE op limited to one embedded wait)
                nc.scalar.activation(
                    vview[:, :, V : 2 * V], ps_view, AF.Copy, bias=1.0, scale=0.0
                )

        # ---- phase 7: attention (software-pipelined over (h, it) groups) ----
        groups = [(h, it) for h in range(H) for it in range(NIT)]
        probs_tiles = {}

        with tc.tile_pool(name="att_sbuf", bufs=2) as attp, tc.tile_pool(
            name="att_psum", bufs=2, space="PSUM"
        ) as attps, tc.tile_pool(name="stat", bufs=2) as statp:

            def emit_scores(g):
                h, it = groups[g]
                ps_s = attps.tile([128, KH], f32, tag="ps_s", bufs=3)
                for jb in range(NJB):
                    nc.tensor.matmul(
                        ps_s[:, jb * 128 : (jb + 1) * 128],
                        lhsT=kf[:, h, jb * 128 : (jb + 1) * 128],
                        rhs=qT[:, h, it * 128 : (it + 1) * 128],
                        start=True,
                        stop=True,
                    )
                pr = attp.tile([128, KH], f16, tag="pr", bufs=3)
                nc.scalar.activation(pr[:], ps_s[:], AF.Exp, bias=0.0, scale=SCALE)
                nc.vector.tensor_mul(pr[:], pr[:], mask_sb[:, it, :, :])
                probs_tiles[g] = pr

            def emit_pv(g):
                h, it = groups[g]
                pr = probs_tiles.pop(g)
                ps_o = attps.tile([128, 128], f32, tag="ps_o")
                for jb in range(NJB):
                    # lhsT columns = [v for head h | ones]: PSUM rows 64:128
                    # come out as the softmax denominator replicated 64x,
                    # partition-aligned with the PV rows for the normalize mul.
                    nc.tensor.matmul(
                        ps_o[:],
                        lhsT=v_sb[:, jb, h * 2 * V : (h + 1) * 2 * V],
                        rhs=pr[:, jb * 128 : (jb + 1) * 128],
                        start=(jb == 0),
                        stop=(jb == NJB - 1),
                    )
                dsum = statp.tile([64, 128], f32, tag="dsum")
                nc.vector.tensor_add(
                    dsum[:], ps_o[64:128, :], bcf(esink_sb[64:128, h : h + 1], 128)
                )
                rcp = statp.tile([64, 128], f32, tag="rcp")
                nc.vector.reciprocal(rcp[:], dsum[:])
                base = (h % 2) * 64
                nc.vector.tensor_mul(
                    oat[base : base + 64, h // 2, it * 128 : (it + 1) * 128],
                    ps_o[0:64, :],
                    rcp[:],
                )
                if DEBUG:
                    nc.sync.dma_start(dbg["pr"][:, g, :], pr[:])

            # two-deep software pipeline: scores run 2 groups ahead of PV so
            # ACT exp + DVE mask latency never stalls the PE stream
            emit_scores(0)
            emit_scores(1)
            for g in range(2, len(groups)):
                emit_scores(g)
                emit_pv(g - 2)
            emit_pv(len(groups) - 2)
            emit_pv(len(groups) - 1)

        # ---- phase 8: output projection (i-major) + bias + store ----
        with tc.tile_pool(name="psf", bufs=1, space="PSUM") as psfp:
            psf = [
                psfp.tile([128, 512], f32, tag=f"psf{i}", name=f"psf{i}")
                for i in range(8)
            ]
            for k in range(8):
                for it in range(NIT):
                    for n in range(4):
                        nc.tensor.matmul(
                            psf[it * 4 + n][:],
                            lhsT=oat[:, k, it * 128 : (it + 1) * 128],
                            rhs=wo_sb[:, k, n * 512 : (n + 1) * 512],
                            start=(k == 0),
                            stop=(k == 7),
                        )
            for it in range(NIT):
                for n in range(4):
                    nc.vector.tensor_add(
                        out_sb[:, it, n * 512 : (n + 1) * 512],
                        psf[it * 4 + n][:],
                        bo_sb[:, n * 512 : (n + 1) * 512],
                    )
                    # SWDGE: first (and only) DMA on each SW queue, so the
                    # ring entry carries just the DVE producer wait.
                    nc.gpsimd.dma_start(
                        out_d[it * 128 : (it + 1) * 128, n * 512 : (n + 1) * 512],
                        out_sb[:, it, n * 512 : (n + 1) * 512],
                    )

        if DEBUG:
            nc.sync.dma_start(dbg["lat"][:], latbf[:])
            nc.sync.dma_start(dbg["lat4"][:], lat4[0:64, :])
            nc.sync.dma_start(dbg["q"][:], qT[:])
            nc.sync.dma_start(dbg["kf"][:], kf[:])
            nc.sync.dma_start(dbg["v"][:], v_sb[:])
            nc.sync.dma_start(dbg["oat"][:], oat[:])

    if not bool(int(os.environ.get("BASSNOSPLIT", "0"))):
        _split_multi_waits(nc, mybir)
    nc.finalize()
    return nc


def _split_multi_waits(nc, mybir):
    """The TPB ISA has a single embedded wait slot per instruction and this
    toolchain's walrus pass list has no wait-splitting pass ("Too many sync
    wait commands"). Hoist all-but-one wait of every multi-wait compute
    instruction into standalone same-engine EventSemaphore instructions
    placed immediately before it. DMA ring entries can't be split this way
    (they don't execute in the engine stream) — the kernel is structured so
    every DMA already has <=1 wait; assert that here."""
    seq_ok = (mybir.InstEventSemaphore,)
    n = 0
    for fn in nc.m.functions:
        for blk in fn.blocks:
            out = []
            for inst in blk.instructions:
                si = inst.sync_info
                if si is not None and len(si.on_wait) > 1 and not isinstance(inst, seq_ok):
                    if isinstance(inst, mybir.InstDMACopy):
                        raise AssertionError(
                            f"DMA {inst.name} has {len(si.on_wait)} waits; "
                            "restructure so DMAs carry at most one"
                        )
                    for w in si.on_wait[:-1]:
                        n += 1
                        out.append(
                            mybir.InstEventSemaphore(
                                name=f"I-wsplit-{n}",
                                engine=inst.engine,
                                ins=[],
                                outs=[],
                                sync_info=mybir.SyncInfo(on_wait=[w], on_update=[]),
                            )
                        )
                    inst.sync_info = mybir.SyncInfo(
                        on_wait=[si.on_wait[-1]], on_update=si.on_update
                    )
                out.append(inst)
            blk.instructions = out
    return n


def prep_inputs(
    hidden_states, cos, sin, Wq, bq, Wo, bo, Wkva, bkva, w_kc, w_vc, sinks
):
    """Build the 8 per-core input dicts (numpy, fp16/fp32)."""
    f16 = np.float16
    hs = np.asarray(hidden_states, np.float32)[0]  # [S, HID]
    cos = np.asarray(cos, np.float32)[0]  # [S, ROPE]
    sin = np.asarray(sin, np.float32)[0]

    wqT = np.ascontiguousarray(np.asarray(Wq, np.float32).T).astype(f16)
    wkvaT = np.ascontiguousarray(np.asarray(Wkva, np.float32).T).astype(f16)
    wkc_p = np.ascontiguousarray(
        np.asarray(w_kc, np.float32).transpose(2, 0, 1).reshape(R, H * NOPE)
    ).astype(f16)
    wvc_p = np.ascontiguousarray(
        np.asarray(w_vc, np.float32).transpose(1, 0, 2).reshape(R, H * V)
    ).astype(f16)
    woT = np.ascontiguousarray(np.asarray(Wo, np.float32).T).astype(f16)

    bq_t = np.ascontiguousarray(np.asarray(bq, np.float32).reshape(16, 128).T)
    bkva_pad = np.zeros(640, np.float32)
    bkva_pad[: R + ROPE] = np.asarray(bkva, np.float32)
    bkva_t = np.ascontiguousarray(bkva_pad.reshape(5, 128).T)
    bo_b = np.ascontiguousarray(
        np.broadcast_to(np.asarray(bo, np.float32), (128, HID))
    ).astype(f16)
    esink_b = np.ascontiguousarray(
        np.broadcast_to(np.exp(np.asarray(sinks, np.float32))[None, :], (128, H))
    )

    hs_pad = np.zeros((SW + S, HID), np.float32)
    hs_pad[SW:] = hs

    shared = dict(
        wqT=wqT, wkvaT=wkvaT, wkc=wkc_p, wvc=wvc_p, woT=woT,
        bq=bq_t, bkva=bkva_t, bo=bo_b, esink=esink_b,
    )

    in_maps = []
    for c in range(NCORES):
        g0 = c * Q
        hsT_c = np.ascontiguousarray(hs_pad[g0 : g0 + KH].T).astype(f16)

        cq = cos[g0 : g0 + Q]  # [Q, 64]
        sq = sin[g0 : g0 + Q]
        tqc = np.zeros((128, Q), np.float32)
        tqs = np.zeros((128, Q), np.float32)
        tqc[64:96] = cq[:, 0:32].T
        tqc[96:128] = cq[:, 32:64].T
        tqs[64:96] = sq[:, 0:32].T
        tqs[96:128] = sq[:, 32:64].T

        kpos = np.clip(np.arange(g0 - SW, g0 + Q), 0, None)
        ck = cos[kpos]  # [KH, 64]
        sk = sin[kpos]
        tkk = np.zeros((64, 2, KH), np.float32)
        tkk[0:32, 0] = ck[:, 0:32].T
        tkk[32:64, 0] = ck[:, 32:64].T
        tkk[0:32, 1] = sk[:, 0:32].T
        tkk[32:64, 1] = sk[:, 32:64].T

        jg = (g0 - SW) + np.arange(KH)  # global key index per (jb, p)
        ig = g0 + np.arange(Q)
        msk = np.zeros((128, NIT, NJB, 128), np.float32)
        for it in range(NIT):
            for jb in range(NJB):
                jj = jg[jb * 128 : (jb + 1) * 128][:, None]  # [128, 1]
                ii = ig[it * 128 : (it + 1) * 128][None, :]  # [1, 128]
                msk[:, it, jb, :] = (
                    (jj >= 0) & (jj <= ii) & (ii - jj < SW)
                ).astype(np.float32)

        in_maps.append(
            dict(
                shared,
                hsT=hsT_c,
                trigq_cos=tqc.astype(f16),
                trigq_sin=tqs.astype(f16),
                trigk=tkk.astype(f16),
                mask=msk.astype(f16),
            )
        )
    return in_maps


def get_program():
    if "nc" not in _CACHE:
        _CACHE["nc"] = _build_program()
    return _CACHE["nc"]


def run(in_maps, **kw):
    from concourse.bass_utils import run_bass_kernel_spmd

    nc = get_program()
    return run_bass_kernel_spmd(nc, in_maps, list(range(NCORES)), **kw)


def kernel(**inputs):
    in_maps = prep_inputs(**inputs)
    res = run(in_maps)
    out = np.concatenate([res.results[c]["out"] for c in range(NCORES)], axis=0)
    return out.reshape(B, S, HID).astype(np.float32)



# revision 5
# speedup vs baseline: 1.4415x; 1.4415x over previous
"""Trainium2 Bass kernel for CareGptOssAttentionHF (MLA-style sliding-window
attention with sinks).

Sharding: sequence-parallel across 8 NeuronCores. Core c owns query rows
[c*256, (c+1)*256) and redundantly computes latent/K/V for its 768-row key
halo [c*256-512, c*256+256) — no collectives needed (window = 512).

On-chip dataflow (per core, all fp16 operands, fp32 PSUM accumulation):
  qT   [d=128, h, i]   = WqT.T @ hsT_local (+bq), RoPE applied in partition dim
  latT [r=576, j]      = WkvaT.T @ hsT_halo (+bkva), then zeroed for halo rows
                         with j<0 (multiplicative jmask) so padded keys vanish
  kfT  [d=128, h, j]   : rows 0:64 = w_kc-projected nope, rows 64:128 = RoPE'd
                         shared k_rope broadcast per head
  v65  [j, h*65+d]     = latT.T @ wvc (key-major; col 64 of each head = jvalid
                         {0,1}, so the q-major PV matmul's column 64 yields the
                         softmax denominator restricted to real keys)
  scoresT[j, i] per h  -> one matmul per (jb) with N=256 covering both query
                         tiles; the 2 universally-masked (it,jb) pairs are
                         skipped -> exp(SCALE*x) on ACT -> {0,1} mask only on
                         the 4 partial diagonal blocks (strided pair APs)
  PV (q-major)         : out[i, v|den] per (h, it); reciprocal on a [128,1]
                         column; normalize via DVE per-partition scalar; oat
                         transposed back to hv-major with 16 PE transposes
  out = oatT.T @ WoT (+bo), stored fp16
"""

import os
import sys

import numpy as np

if "/opt/trn_rl_repo" not in sys.path:
    sys.path.insert(0, "/opt/trn_rl_repo")

B, S, HID, H = 1, 2048, 2048, 16
NOPE = ROPE = 64
D = NOPE + ROPE  # 128
V = 64
R = 512
SW = 512
NCORES = 8
Q = S // NCORES  # 256 query rows per core
KH = Q + SW  # 768 halo key rows per core
NJB = KH // 128  # 6 key blocks
NIT = Q // 128  # 2 query tiles
SCALE = float(D) ** -0.5
DEBUG = bool(int(os.environ.get("BASSDBG", "0")))

# valid jb sets per query tile: (it0, jb5) has j>i always, (it1, jb0) has
# i-j>=512 always -> both skipped on every core
JB_IT = (tuple(range(0, 5)), tuple(range(1, 6)))
# (it, jb) pairs needing a partial causal/window mask, in mask_sb slot order
MASK_SLOTS = ((0, 0), (0, 4), (1, 1), (1, 5))

_CACHE = {}


def _build_program():
    import concourse.bass as bass
    import concourse.mybir as mybir
    from concourse import tile
    from contextlib import ExitStack

    f32 = mybir.dt.float32
    f16 = mybir.dt.float16
    AF = mybir.ActivationFunctionType
    OP = mybir.AluOpType

    nc = bass.Bass()

    hsT_d = nc.dram_tensor("hsT", [HID, KH], f16, kind="ExternalInput")
    wqT_d = nc.dram_tensor("wqT", [HID, H * D], f16, kind="ExternalInput")
    wkvaT_d = nc.dram_tensor("wkvaT", [HID, R + ROPE], f16, kind="ExternalInput")
    wkc_d = nc.dram_tensor("wkc", [R, H * NOPE], f16, kind="ExternalInput")
    wvc_d = nc.dram_tensor("wvc", [R, H * V], f16, kind="ExternalInput")
    woT_d = nc.dram_tensor("woT", [H * V, HID], f16, kind="ExternalInput")
    bq_d = nc.dram_tensor("bq", [128, 16], f32, kind="ExternalInput")
    bkva_d = nc.dram_tensor("bkva", [128, 5], f32, kind="ExternalInput")
    bo_d = nc.dram_tensor("bo", [128, HID], f16, kind="ExternalInput")
    tqc_d = nc.dram_tensor("trigq_cos", [128, Q], f16, kind="ExternalInput")
    tqs_d = nc.dram_tensor("trigq_sin", [128, Q], f16, kind="ExternalInput")
    tk_d = nc.dram_tensor("trigk", [64, 2, KH], f16, kind="ExternalInput")
    mask_d = nc.dram_tensor("mask4", [128, 4, 128], f16, kind="ExternalInput")
    jmask_d = nc.dram_tensor("jmask", [128, KH], f16, kind="ExternalInput")
    jvalid_d = nc.dram_tensor("jvalid", [128, NJB], f16, kind="ExternalInput")
    ident_d = nc.dram_tensor("ident", [128, 128], f16, kind="ExternalInput")
    esink_d = nc.dram_tensor("esink", [128, H], f32, kind="ExternalInput")
    out_d = nc.dram_tensor("out", [Q, HID], f16, kind="ExternalOutput")

    dbg = {}
    if DEBUG:
        dbg["lat"] = nc.dram_tensor("dbg_lat", [128, 4, KH], f16, kind="ExternalOutput")
        dbg["lat4"] = nc.dram_tensor("dbg_lat4", [64, KH], f16, kind="ExternalOutput")
        dbg["q"] = nc.dram_tensor("dbg_q", [128, H, Q], f16, kind="ExternalOutput")
        dbg["kf"] = nc.dram_tensor("dbg_kf", [128, H, KH], f16, kind="ExternalOutput")
        dbg["v"] = nc.dram_tensor("dbg_v", [128, NJB, H * 65], f16, kind="ExternalOutput")
        dbg["pr"] = nc.dram_tensor("dbg_pr", [128, H, NJB, Q], f16, kind="ExternalOutput")
        dbg["oatq"] = nc.dram_tensor("dbg_oatq", [128, NIT, H * V], f16, kind="ExternalOutput")
        dbg["oat"] = nc.dram_tensor("dbg_oat", [128, 8, Q], f16, kind="ExternalOutput")

    with tile.TileContext(nc) as tc, ExitStack() as ctx:
        const = ctx.enter_context(tc.tile_pool(name="const", bufs=1))

        # ---- resident tiles (DMAs issued below in ring order) ----
        hs_t = [const.tile([128, KH], f16, name=f"hs{k}") for k in range(16)]
        wkva = const.tile([128, 16, R + ROPE], f16)
        wkc = const.tile([128, 4, H * NOPE], f16)
        wvc = const.tile([128, 4, H * V], f16)
        bq_sb = const.tile([128, 16], f32)
        bkva_sb = const.tile([128, 5], f32)
        bo_sb = const.tile([128, HID], f16)
        tqc = const.tile([128, Q], f16)
        tqs = const.tile([128, Q], f16)
        tk = const.tile([64, 2, KH], f16)
        mask_sb = const.tile([128, 4, 128], f16)
        jmask_sb = const.tile([128, KH], f16)
        jvalid_sb = const.tile([128, NJB], f16)
        ident_sb = const.tile([128, 128], f16)
        esink_sb = const.tile([128, H], f32)
        wo_sb = const.tile([128, 8, HID], f16)

        qT = const.tile([128, H, Q], f16)
        latbf = const.tile([128, 4, KH], f16)
        lat4 = const.tile([64, KH], f16)
        oatq = const.tile([128, NIT, H * V], f16)
        out_sb = const.tile([128, NIT, HID], f16)

        def bc(ap, n):
            # broadcast a [P, F] AP to [P, n, F] via a step-0 middle dim
            return bass.AP(ap.tensor, ap.offset, [ap.ap[0], [0, n], ap.ap[1]])

        # ---- phase 1: q projection, DMA-paced on interleaved hs/wq tiles ----
        # Wq is loaded whole into an 8MB region with first-use (zero-wait)
        # DMAs; the region is released afterward and reused by kf/v65/etc,
        # whose writers are compute engines (multi-wait capable), never DMAs —
        # DMA ring entries can carry at most ONE embedded wait.
        with tc.tile_pool(name="wqfull", bufs=1) as wqp, tc.tile_pool(
            name="psq", bufs=1, space="PSUM"
        ) as psqp:
            wq_t = [wqp.tile([128, H * D], f16, name=f"wq{k}") for k in range(16)]
            # ring order = arrival order: the pair (hs_k, wq_k) unblocks the
            # k-th contraction step of phase 1 ~2us apart, so the PE starts
            # ~2us into the kernel instead of after the whole input load.
            for k in range(16):
                nc.sync.dma_start(hs_t[k][:], hsT_d[k * 128 : (k + 1) * 128, :])
                nc.sync.dma_start(wq_t[k][:], wqT_d[k * 128 : (k + 1) * 128, :])
            for k in range(16):
                nc.sync.dma_start(wkva[:, k, :], wkvaT_d[k * 128 : (k + 1) * 128, :])
            for k in range(4):
                nc.sync.dma_start(wkc[:, k, :], wkc_d[k * 128 : (k + 1) * 128, :])
            for k in range(4):
                nc.sync.dma_start(wvc[:, k, :], wvc_d[k * 128 : (k + 1) * 128, :])
            nc.sync.dma_start(bq_sb[:], bq_d[:])
            nc.sync.dma_start(bkva_sb[:], bkva_d[:])
            nc.sync.dma_start(tqc[:], tqc_d[:])
            nc.sync.dma_start(tqs[:], tqs_d[:])
            nc.sync.dma_start(tk[:], tk_d[:])
            nc.sync.dma_start(mask_sb[:], mask_d[:])
            nc.sync.dma_start(jmask_sb[:], jmask_d[:])
            nc.sync.dma_start(jvalid_sb[:], jvalid_d[:])
            nc.sync.dma_start(ident_sb[:], ident_d[:])
            nc.sync.dma_start(esink_sb[:], esink_d[:])
            nc.sync.dma_start(bo_sb[:], bo_d[:])
            # loaded last: first consumed by the final output projection
            for k in range(8):
                nc.sync.dma_start(wo_sb[:, k, :], woT_d[k * 128 : (k + 1) * 128, :])

            for g in range(2):
                psq = [
                    psqp.tile([128, Q], f32, tag=f"psq{m}", name=f"psq{m}")
                    for m in range(8)
                ]
                for k in range(16):
                    for m in range(8):
                        fo = g * 1024 + m * 128
                        nc.tensor.matmul(
                            psq[m][:],
                            lhsT=wq_t[k][:, fo : fo + 128],
                            rhs=hs_t[k][:, SW:KH],
                            start=(k == 0),
                            stop=(k == 15),
                        )
                for m in range(8):
                    gm = g * 8 + m
                    nc.vector.tensor_scalar(
                        qT[:, gm, :], psq[m][:], bq_sb[:, gm : gm + 1], None, OP.add
                    )

        # long-lived tiles reusing the released Wq region; writers are all
        # compute engines (ACT/DVE/Pool), never DMAs.
        kvp = ctx.enter_context(tc.tile_pool(name="kv", bufs=1))
        kf = kvp.tile([128, H, KH], f16)
        # per-head column layout [v(64) | jvalid(1)] so the q-major PV
        # matmul's output column 64 is the masked softmax denominator
        v65 = kvp.tile([128, NJB, H * 65], f16)
        rotq = kvp.tile([128, 8, Q], f16)
        rotk = kvp.tile([64, KH], f16)
        oat = kvp.tile([128, 8, Q], f16)

        # ---- phase 2: RoPE on q (rows 64:128 of each head tile) ----
        for hb in range(2):
            hs_ = slice(hb * 8, hb * 8 + 8)
            nc.vector.tensor_copy(rotq[64:96, :, :], qT[96:128, hs_, :])
            nc.vector.tensor_copy(rotq[96:128, :, :], qT[64:96, hs_, :])
            nc.vector.tensor_mul(qT[64:96, hs_, :], qT[64:96, hs_, :], bc(tqc[64:96, :], 8))
            nc.vector.tensor_mul(rotq[64:96, :, :], rotq[64:96, :, :], bc(tqs[64:96, :], 8))
            nc.vector.tensor_sub(qT[64:96, hs_, :], qT[64:96, hs_, :], rotq[64:96, :, :])
            nc.vector.tensor_mul(qT[96:128, hs_, :], qT[96:128, hs_, :], bc(tqc[96:128, :], 8))
            nc.vector.tensor_mul(rotq[96:128, :, :], rotq[96:128, :, :], bc(tqs[96:128, :], 8))
            nc.vector.tensor_add(qT[96:128, hs_, :], qT[96:128, hs_, :], rotq[96:128, :, :])

        # ---- phase 3: latent projection (+bias, then zero j<0 halo rows) ----
        with tc.tile_pool(name="pslat", bufs=1, space="PSUM") as pslatp:
            pslat = [
                pslatp.tile([128, KH], f32, tag=f"pslat{m}", name=f"pslat{m}")
                for m in range(4)
            ]
            for k in range(16):
                for m in range(4):
                    for n0, n1 in ((0, 512), (512, KH)):
                        nc.tensor.matmul(
                            pslat[m][:, n0:n1],
                            lhsT=wkva[:, k, m * 128 : (m + 1) * 128],
                            rhs=hs_t[k][:, n0:n1],
                            start=(k == 0),
                            stop=(k == 15),
                        )
            for m in range(4):
                # latbf = (psum + bkva) * jmask : rows with j<0 become exactly
                # zero so padded halo keys contribute nothing anywhere
                nc.vector.scalar_tensor_tensor(
                    latbf[:, m, :], pslat[m][:], bkva_sb[:, m : m + 1],
                    jmask_sb[:], OP.add, OP.mult,
                )
            ps4 = pslatp.tile([64, KH], f32, tag="pslat0")
            for k in range(16):
                for n0, n1 in ((0, 512), (512, KH)):
                    nc.tensor.matmul(
                        ps4[:, n0:n1],
                        lhsT=wkva[:, k, 512:576],
                        rhs=hs_t[k][:, n0:n1],
                        start=(k == 0),
                        stop=(k == 15),
                    )
            nc.vector.scalar_tensor_tensor(
                lat4[:], ps4[:], bkva_sb[0:64, 4:5], jmask_sb[0:64, :],
                OP.add, OP.mult,
            )

        # ---- phase 4: RoPE on k_rope + broadcast into kf rows 64:128 ----
        nc.vector.tensor_copy(rotk[0:32, :], lat4[32:64, :])
        nc.vector.tensor_copy(rotk[32:64, :], lat4[0:32, :])
        nc.vector.tensor_mul(lat4[0:32, :], lat4[0:32, :], tk[0:32, 0, :])
        nc.vector.tensor_mul(rotk[0:32, :], rotk[0:32, :], tk[0:32, 1, :])
        nc.vector.tensor_sub(lat4[0:32, :], lat4[0:32, :], rotk[0:32, :])
        nc.vector.tensor_mul(lat4[32:64, :], lat4[32:64, :], tk[32:64, 0, :])
        nc.vector.tensor_mul(rotk[32:64, :], rotk[32:64, :], tk[32:64, 1, :])
        nc.vector.tensor_add(lat4[32:64, :], lat4[32:64, :], rotk[32:64, :])
        # 16 narrow SBUF->SBUF broadcast copies, all on Pool (it cannot touch
        # PSUM, so this is the one copy job it can own outright)
        for h in range(H):
            nc.gpsimd.tensor_copy(kf[64:128, h, :], lat4[:])

        # ---- phase 5: k_nope into kf rows 0:64 ----
        with tc.tile_pool(name="pskn", bufs=4, space="PSUM") as psknp:
            for m in range(8):
                ps = psknp.tile([128, KH], f32, tag="pskn")
                for k in range(4):
                    for n0, n1 in ((0, 512), (512, KH)):
                        nc.tensor.matmul(
                            ps[:, n0:n1],
                            lhsT=wkc[:, k, m * 128 : (m + 1) * 128],
                            rhs=latbf[:, k, n0:n1],
                            start=(k == 0),
                            stop=(k == 3),
                        )
                # PSUM drains split ACT/DVE (Pool cannot read PSUM)
                nc.scalar.copy(kf[0:64, 2 * m, :], ps[0:64, :])
                nc.vector.tensor_copy(kf[0:64, 2 * m + 1, :], ps[64:128, :])

        # ---- phase 6: V (key-major, jvalid column per head) ----
        with tc.tile_pool(name="psv", bufs=2, space="PSUM") as psvp:
            for jb in range(NJB):
                ps = psvp.tile([128, H * V], f32, tag="psv")
                for k in range(4):
                    for n0, n1 in ((0, 512), (512, 1024)):
                        nc.tensor.matmul(
                            ps[:, n0:n1],
                            lhsT=latbf[:, k, jb * 128 : (jb + 1) * 128],
                            rhs=wvc[:, k, n0:n1],
                            start=(k == 0),
                            stop=(k == 3),
                        )
                vview = v65[:, jb, :].rearrange("p (h d) -> p h d", d=65)
                ps_view = ps[:].rearrange("p (h d) -> p h d", d=V)
                nc.scalar.copy(vview[:, :, 0:V], ps_view)
                nc.scalar.copy(vview[:, :, V : V + 1], bc(jvalid_sb[:, jb : jb + 1], H))

        # ---- phase 7: attention (software-pipelined over heads) ----
        probs_tiles = {}

        with tc.tile_pool(name="att_sbuf", bufs=2) as attp, tc.tile_pool(
            name="att_psum", bufs=2, space="PSUM"
        ) as attps, tc.tile_pool(name="stat", bufs=4) as statp:

            def emit_scores(h):
                ps_s = attps.tile([128, NJB, 256], f32, tag="ps_s")
                # jb0 is only valid for query tile 0, jb5 only for tile 1;
                # jb1..4 cover both tiles in one N=256 matmul
                nc.tensor.matmul(
                    ps_s[:, 0, 0:128],
                    lhsT=kf[:, h, 0:128],
                    rhs=qT[:, h, 0:128],
                    start=True, stop=True,
                )
                for jb in range(1, 5):
                    nc.tensor.matmul(
                        ps_s[:, jb, :],
                        lhsT=kf[:, h, jb * 128 : (jb + 1) * 128],
                        rhs=qT[:, h, :],
                        start=True, stop=True,
                    )
                nc.tensor.matmul(
                    ps_s[:, 5, 128:256],
                    lhsT=kf[:, h, 640:768],
                    rhs=qT[:, h, 128:256],
                    start=True, stop=True,
                )
                pr = attp.tile([128, NJB, 256], f16, tag="pr", bufs=3)
                # single full-rect exp; the two never-written corner blocks
                # (jb5/it0, jb0/it1) may exp stale PSUM but are never read
                nc.scalar.activation(pr[:], ps_s[:], AF.Exp, bias=0.0, scale=SCALE)
                # partial causal/window masks: (it0: jb0,jb4), (it1: jb1,jb5),
                # each pair as one strided DVE op
                nc.vector.tensor_mul(
                    pr[:, 0:5:4, 0:128], pr[:, 0:5:4, 0:128], mask_sb[:, 0:2, :]
                )
                nc.vector.tensor_mul(
                    pr[:, 1:6:4, 128:256], pr[:, 1:6:4, 128:256], mask_sb[:, 2:4, :]
                )
                probs_tiles[h] = pr
                if DEBUG:
                    nc.sync.dma_start(dbg["pr"][:, h, :, :], pr[:])

            def emit_pv(h):
                pr = probs_tiles.pop(h)
                for it in range(NIT):
                    jbs = JB_IT[it]
                    ps_o = attps.tile([128, 65], f32, tag="ps_o")
                    for n, jb in enumerate(jbs):
                        nc.tensor.matmul(
                            ps_o[:],
                            lhsT=pr[:, jb, it * 128 : (it + 1) * 128],
                            rhs=v65[:, jb, h * 65 : (h + 1) * 65],
                            start=(n == 0),
                            stop=(n == len(jbs) - 1),
                        )
                    dsc = statp.tile([128, 1], f32, tag="dsc")
                    nc.vector.tensor_scalar(
                        dsc[:], ps_o[:, 64:65], esink_sb[:, h : h + 1], None, OP.add
                    )
                    rcp = statp.tile([128, 1], f32, tag="rcp")
                    nc.vector.reciprocal(rcp[:], dsc[:])
                    nc.vector.tensor_scalar(
                        oatq[:, it, h * V : (h + 1) * V],
                        ps_o[:, 0:V], rcp[:], None, OP.mult,
                    )

            emit_scores(0)
            emit_scores(1)
            for h in range(2, H):
                emit_scores(h)
                emit_pv(h - 2)
            emit_pv(H - 2)
            emit_pv(H - 1)

        # ---- phase 7b: transpose oat back to hv-major for the out proj ----
        with tc.tile_pool(name="pst", bufs=2, space="PSUM") as pstp:
            for m in range(8):
                for it in range(NIT):
                    pst = pstp.tile([128, 128], f16, tag="pst")
                    nc.tensor.transpose(
                        pst[:], oatq[:, it, m * 128 : (m + 1) * 128], ident_sb[:]
                    )
                    eng = nc.scalar if it == 0 else nc.vector
                    if eng is nc.scalar:
                        eng.copy(oat[:, m, it * 128 : (it + 1) * 128], pst[:])
                    else:
                        eng.tensor_copy(oat[:, m, it * 128 : (it + 1) * 128], pst[:])

        # ---- phase 8: output projection (i-major) + bias + store ----
        with tc.tile_pool(name="psf", bufs=1, space="PSUM") as psfp:
            psf = [
                psfp.tile([128, 512], f32, tag=f"psf{i}", name=f"psf{i}")
                for i in range(8)
            ]
            for k in range(8):
                for it in range(NIT):
                    for n in range(4):
                        nc.tensor.matmul(
                            psf[it * 4 + n][:],
                            lhsT=oat[:, k, it * 128 : (it + 1) * 128],
                            rhs=wo_sb[:, k, n * 512 : (n + 1) * 512],
                            start=(k == 0),
                            stop=(k == 7),
                        )
            for it in range(NIT):
                for n in range(4):
                    nc.vector.tensor_add(
                        out_sb[:, it, n * 512 : (n + 1) * 512],
                        psf[it * 4 + n][:],
                        bo_sb[:, n * 512 : (n + 1) * 512],
                    )
                    # SWDGE: each ring entry carries just the DVE producer wait
                    nc.gpsimd.dma_start(
                        out_d[it * 128 : (it + 1) * 128, n * 512 : (n + 1) * 512],
                        out_sb[:, it, n * 512 : (n + 1) * 512],
                    )

        if DEBUG:
            nc.sync.dma_start(dbg["lat"][:], latbf[:])
            nc.sync.dma_start(dbg["lat4"][:], lat4[:])
            nc.sync.dma_start(dbg["q"][:], qT[:])
            nc.sync.dma_start(dbg["kf"][:], kf[:])
            nc.sync.dma_start(dbg["v"][:], v65[:])
            nc.sync.dma_start(dbg["oatq"][:], oatq[:])
            nc.sync.dma_start(dbg["oat"][:], oat[:])

    if not bool(int(os.environ.get("BASSNOSPLIT", "0"))):
        _split_multi_waits(nc, mybir)
    nc.finalize()
    return nc


def _split_multi_waits(nc, mybir):
    """The TPB ISA has a single embedded wait slot per instruction and this
    toolchain's walrus pass list has no wait-splitting pass ("Too many sync
    wait commands"). Hoist all-but-one wait of every multi-wait compute
    instruction into standalone same-engine EventSemaphore instructions
    placed immediately before it. DMA ring entries can't be split this way
    (they don't execute in the engine stream) — the kernel is structured so
    every DMA already has <=1 wait; assert that here."""
    seq_ok = (mybir.InstEventSemaphore,)
    n = 0
    for fn in nc.m.functions:
        for blk in fn.blocks:
            out = []
            for inst in blk.instructions:
                si = inst.sync_info
                if si is not None and len(si.on_wait) > 1 and not isinstance(inst, seq_ok):
                    if isinstance(inst, mybir.InstDMACopy):
                        raise AssertionError(
                            f"DMA {inst.name} has {len(si.on_wait)} waits; "
                            "restructure so DMAs carry at most one"
                        )
                    for w in si.on_wait[:-1]:
                        n += 1
                        out.append(
                            mybir.InstEventSemaphore(
                                name=f"I-wsplit-{n}",
                                engine=inst.engine,
                                ins=[],
                                outs=[],
                                sync_info=mybir.SyncInfo(on_wait=[w], on_update=[]),
                            )
                        )
                    inst.sync_info = mybir.SyncInfo(
                        on_wait=[si.on_wait[-1]], on_update=si.on_update
                    )
                out.append(inst)
            blk.instructions = out
    return n


def prep_inputs(
    hidden_states, cos, sin, Wq, bq, Wo, bo, Wkva, bkva, w_kc, w_vc, sinks
):
    """Build the 8 per-core input dicts (numpy, fp16/fp32)."""
    f16 = np.float16
    hs = np.asarray(hidden_states, np.float32)[0]  # [S, HID]
    cos = np.asarray(cos, np.float32)[0]  # [S, ROPE]
    sin = np.asarray(sin, np.float32)[0]

    wqT = np.ascontiguousarray(np.asarray(Wq, np.float32).T).astype(f16)
    wkvaT = np.ascontiguousarray(np.asarray(Wkva, np.float32).T).astype(f16)
    wkc_p = np.ascontiguousarray(
        np.asarray(w_kc, np.float32).transpose(2, 0, 1).reshape(R, H * NOPE)
    ).astype(f16)
    wvc_p = np.ascontiguousarray(
        np.asarray(w_vc, np.float32).transpose(1, 0, 2).reshape(R, H * V)
    ).astype(f16)
    woT = np.ascontiguousarray(np.asarray(Wo, np.float32).T).astype(f16)

    bq_t = np.ascontiguousarray(np.asarray(bq, np.float32).reshape(16, 128).T)
    bkva_pad = np.zeros(640, np.float32)
    bkva_pad[: R + ROPE] = np.asarray(bkva, np.float32)
    bkva_t = np.ascontiguousarray(bkva_pad.reshape(5, 128).T)
    bo_b = np.ascontiguousarray(
        np.broadcast_to(np.asarray(bo, np.float32), (128, HID))
    ).astype(f16)
    esink_b = np.ascontiguousarray(
        np.broadcast_to(np.exp(np.asarray(sinks, np.float32))[None, :], (128, H))
    )
    ident = np.eye(128, dtype=f16)

    hs_pad = np.zeros((SW + S, HID), np.float32)
    hs_pad[SW:] = hs

    shared = dict(
        wqT=wqT, wkvaT=wkvaT, wkc=wkc_p, wvc=wvc_p, woT=woT,
        bq=bq_t, bkva=bkva_t, bo=bo_b, esink=esink_b, ident=ident,
    )

    in_maps = []
    for c in range(NCORES):
        g0 = c * Q
        hsT_c = np.ascontiguousarray(hs_pad[g0 : g0 + KH].T).astype(f16)

        cq = cos[g0 : g0 + Q]  # [Q, 64]
        sq = sin[g0 : g0 + Q]
        tqc = np.zeros((128, Q), np.float32)
        tqs = np.zeros((128, Q), np.float32)
        tqc[64:96] = cq[:, 0:32].T
        tqc[96:128] = cq[:, 32:64].T
        tqs[64:96] = sq[:, 0:32].T
        tqs[96:128] = sq[:, 32:64].T

        kpos = np.clip(np.arange(g0 - SW, g0 + Q), 0, None)
        ck = cos[kpos]  # [KH, 64]
        sk = sin[kpos]
        tkk = np.zeros((64, 2, KH), np.float32)
        tkk[0:32, 0] = ck[:, 0:32].T
        tkk[32:64, 0] = ck[:, 32:64].T
        tkk[0:32, 1] = sk[:, 0:32].T
        tkk[32:64, 1] = sk[:, 32:64].T

        jg = (g0 - SW) + np.arange(KH)  # global key index per (jb, p)
        ig = g0 + np.arange(Q)
        msk = np.zeros((128, 4, 128), np.float32)
        for s_, (it, jb) in enumerate(MASK_SLOTS):
            jj = jg[jb * 128 : (jb + 1) * 128][:, None]  # [128, 1]
            ii = ig[it * 128 : (it + 1) * 128][None, :]  # [1, 128]
            msk[:, s_, :] = (
                (jj >= 0) & (jj <= ii) & (ii - jj < SW)
            ).astype(np.float32)
        jmask = np.ascontiguousarray(
            np.broadcast_to((jg >= 0).astype(np.float32)[None, :], (128, KH))
        )
        jvalid = (jg.reshape(NJB, 128).T >= 0).astype(np.float32)  # [128, NJB]

        in_maps.append(
            dict(
                shared,
                hsT=hsT_c,
                trigq_cos=tqc.astype(f16),
                trigq_sin=tqs.astype(f16),
                trigk=tkk.astype(f16),
                mask4=msk.astype(f16),
                jmask=jmask.astype(f16),
                jvalid=jvalid.astype(f16),
            )
        )
    return in_maps


def get_program():
    if "nc" not in _CACHE:
        _CACHE["nc"] = _build_program()
    return _CACHE["nc"]


def run(in_maps, **kw):
    from concourse.bass_utils import run_bass_kernel_spmd

    nc = get_program()
    return run_bass_kernel_spmd(nc, in_maps, list(range(NCORES)), **kw)


def kernel(**inputs):
    in_maps = prep_inputs(**inputs)
    res = run(in_maps)
    out = np.concatenate([res.results[c]["out"] for c in range(NCORES)], axis=0)
    return out.reshape(B, S, HID).astype(np.float32)
